# revision 1
# baseline (speedup 1.0000x reference)
"""MLA (multi-head latent attention) Trainium2 kernel, SPMD over 8 NeuronCores.

Sharding: core c = 4*b + j handles batch b and query rows [512j, 512j+512).
Each core computes the full K/V path for its batch (replicated within the
4-core batch group), attention + out-projection for its own query chunk.
No collectives; the host gather is a pure concat of disjoint output chunks.

All on-chip tensors live in transposed ([feature, token]) layouts so every
matmul contracts over the partition dim with no on-chip transposes:
  kv_latT[l,t] -> kcT/vT per head -> scoresT[k,q] -> exp -> ctxT[d,q] -> out[t,o]
rotate_half is folded into host-permuted weight copies; 1/sqrt(dh) into the
q weights; softmax skips the max-pass (scores bounded, exp cannot overflow)
and gets its row-sum from an all-ones matmul; normalization is fused into
the ctx PSUM evacuation.
"""

import contextlib
import os
import sys
import types

for _p in ("/opt/trn_rl_repo", "/root/.axon_site/_ro/trn_rl_repo"):
    if os.path.isdir(_p) and _p not in sys.path:
        sys.path.append(_p)

import numpy as np
import ml_dtypes

import concourse.bass as bass
import concourse.bacc as bacc_mod
import concourse.mybir as mybir
from concourse.tile import TileContext
from concourse.vector_clock import ScopedClock
from concourse.bass_utils import run_bass_kernel_spmd

F32 = mybir.dt.float32
BF16 = mybir.dt.bfloat16
BF16NP = ml_dtypes.bfloat16

HID, H, LAT, R, DH, C = 2048, 16, 512, 32, 128, 96
B, S = 2, 2048
SQ = 512          # query rows per core
NKC = S // 128    # 16 key chunks of 128
NG, GH = 4, 4     # 4 head-groups of 4 heads


def _patch_tile_drain():
    """The staged walrus rejects a Drain carrying >1 sync-wait. Move the
    TileContext tail-drain waits onto single-wait SP nops."""

    def _drain_and_barrier(self, tick_clock, wait_clock):
        drain_inst = self.nc.sync.drain()
        wait_clock.add_sem_waits(
            drain_inst.ins, ScopedClock({None: tick_clock.global_clock})
        )
        si = drain_inst.ins.sync_info
        if si is not None and len(si.on_wait) > 1:
            waits = list(si.on_wait)
            drain_inst.ins.sync_info = mybir.SyncInfo(
                on_wait=[], on_update=list(si.on_update)
            )
            for w in waits:
                nop = self.nc.sync.nop(nofuse=True)
                nop.ins.sync_info = mybir.SyncInfo(on_wait=[w], on_update=[])
        self.nc.all_engine_barrier()
        assert self.sems is not None
        popped = self.nc._tile_sem_poison_stack.pop()
        assert popped is self._sem_poison
        self.nc.clear_and_free_semaphores(list(self.sems.allocated().values()))
        self.nc.all_engine_barrier()

    TileContext._drain_and_barrier = _drain_and_barrier


def _install_ntff_hook():
    """antenv.axon_hooks is absent in this image; inject it and register the
    ctypes NTFF hook so trace=True / BASS_TRACE can profile."""
    try:
        import antenv

        if "antenv.axon_hooks" not in sys.modules:
            mod = types.ModuleType("antenv.axon_hooks")
            mod._hook = None

            def set_axon_ntff_profile_hook(h):
                mod._hook = h

            def get_axon_ntff_profile_hook():
                return mod._hook

            mod.set_axon_ntff_profile_hook = set_axon_ntff_profile_hook
            mod.get_axon_ntff_profile_hook = get_axon_ntff_profile_hook
            sys.modules["antenv.axon_hooks"] = mod
            antenv.axon_hooks = mod
        boot_dir = "/root/.axon_site/trn_agent_boot"
        so_path = "/opt/axon/libaxon_pjrt.so"
        if os.path.isdir(boot_dir) and os.path.exists(so_path):
            if boot_dir not in sys.path:
                sys.path.append(boot_dir)
            from trn_boot import _ntff_profile_via_ctypes

            hook = _ntff_profile_via_ctypes(so_path)
            if hook is not None:
                sys.modules["antenv.axon_hooks"].set_axon_ntff_profile_hook(hook)
    except Exception:
        pass


_patch_tile_drain()
_install_ntff_hook()


def _dram(nc, name, shape, dtype=F32, out=False):
    return nc.declare_dram_parameter(name, list(shape), dtype, isOutput=out)


def build_nc():
    nc = bacc_mod.Bacc("TRN2")

    xbT = _dram(nc, "xbT", [HID, S], BF16)            # x[b].T
    xqT = _dram(nc, "xqT", [HID, SQ], BF16)           # query-chunk rows of x[b], T
    wd_kvT = _dram(nc, "wd_kvT", [HID, LAT], BF16)    # Wkv_down.T
    wd_qT = _dram(nc, "wd_qT", [HID, LAT], BF16)      # Wq_down.T
    wkrT = _dram(nc, "wkrT", [HID, R], BF16)          # Wk_rope.T
    wkrrT = _dram(nc, "wkrrT", [HID, R], BF16)        # rot(Wk_rope).T
    wk_upT = _dram(nc, "wk_upT", [LAT, C * H], BF16)  # Wk_up.T
    wv_upT = _dram(nc, "wv_upT", [LAT, HID], BF16)    # Wv_up.T
    wqcrT = _dram(nc, "wqcrT", [LAT, 128 * H], BF16)  # per head: [Wq_up_h; Wq_rope_h].T / sqrt(DH)
    wqrrT = _dram(nc, "wqrrT", [LAT, 2 * R * H], BF16)    # per head: rot(Wq_rope_h).T / sqrt(DH)
    woT = _dram(nc, "woT", [HID, HID], BF16)    # Wo.T (bf16)
    bo_d = _dram(nc, "bo", [128, HID])  # host-broadcast
    cosqT_d = _dram(nc, "cosqT", [R, SQ])
    sinqT_d = _dram(nc, "sinqT", [R, SQ])
    coskT_d = _dram(nc, "coskT", [R, S], BF16)
    sinkT_d = _dram(nc, "sinkT", [R, S], BF16)
    maskT_d = _dram(nc, "maskT", [128, NKC * SQ], BF16)
    out_d = _dram(nc, "out", [SQ, HID], out=True)

    # [ (hc p) f ] views with 128-partition tiling of the contraction dim
    xbT_r2 = xbT[:, :].rearrange("(c p two) t -> c p two t", p=128, two=2)
    xqT_r2 = xqT[:, :].rearrange("(c p two) t -> c p two t", p=128, two=2)
    wd_kvT_r2 = wd_kvT[:, :].rearrange("(c p two) l -> c p two l", p=128, two=2)
    wd_qT_r2 = wd_qT[:, :].rearrange("(c p two) l -> c p two l", p=128, two=2)
    wkrT_r2 = wkrT[:, :].rearrange("(c p two) r -> c p two r", p=128, two=2)
    wkrrT_r2 = wkrrT[:, :].rearrange("(c p two) r -> c p two r", p=128, two=2)
    wk_upT_r = wk_upT[:, :].rearrange("(lc p) c -> lc p c", p=128)
    wv_upT_r = wv_upT[:, :].rearrange("(lc p) d -> lc p d", p=128)
    wqcrT_r = wqcrT[:, :].rearrange("(lc p) d -> lc p d", p=128)
    wqrrT_r = wqrrT[:, :].rearrange("(lc p) d -> lc p d", p=128)
    woT_r = woT[:, :].rearrange("(hc p) o -> hc p o", p=128)

    with TileContext(nc) as tc:
        with tc.tile_pool(name="perB", bufs=1) as perB:
            WO = None  # opened before phase 2; closed with perB
            ctxT = perB.tile([128, H, SQ], BF16, tag="ctxT", name="ctxT")  # [d%128, h, q]
            cosqT = perB.tile([32, SQ], F32, tag="cosq", name="cosq")
            sinqT = perB.tile([32, SQ], F32, tag="sinq", name="sinq")
            onesb = perB.tile([128, 128], BF16, tag="ones", name="ones")
            nc.sync.dma_start(cosqT[:], cosqT_d[:, :])
            nc.sync.dma_start(sinqT[:], sinqT_d[:, :])
            nc.gpsimd.memset(onesb[:], 1.0)

            with tc.tile_pool(name="perA", bufs=1) as perA:
                kv_latT = perA.tile([128, 4, S], BF16, tag="kvlat", name="kvlat")
                q_latT = perA.tile([128, 4, SQ], BF16, tag="qlat", name="qlat")
                krT = perA.tile([32, S], BF16, tag="krT", name="krT")
                maskT = perA.tile([128, NKC * SQ], BF16, tag="mask", name="mask")

                # ------------ Phase 1: latents + roped k_rope ------------
                with tc.tile_pool(name="w1", bufs=1) as W1, \
                     tc.tile_pool(name="xs", bufs=3) as XS, \
                     tc.tile_pool(name="tmp1", bufs=3) as T1, \
                     tc.tile_pool(name="ps_lat", bufs=1, space="PSUM") as PSL:
                    coskT = W1.tile([32, S], BF16, tag="cosk", name="cosk")
                    sinkT = W1.tile([32, S], BF16, tag="sink", name="sink")
                    wdkv = [W1.tile([128, 2, LAT], BF16, tag=f"wdkv{hch}",
                                    name=f"wdkv{hch}") for hch in range(8)]
                    wdq = [W1.tile([128, 2, LAT], BF16, tag=f"wdq{hch}",
                                   name=f"wdq{hch}") for hch in range(8)]
                    wkr = [W1.tile([128, 2, 2 * R], BF16, tag=f"wkr{hch}",
                                   name=f"wkr{hch}") for hch in range(8)]

                    # kv_latT + k_rope in one pass over xbT t-quarters
                    for tq in range(4):
                        tsl = slice(tq * 512, (tq + 1) * 512)
                        pss = [PSL.tile([128, 512], F32, tag=f"lat{lc}_0",
                                        name=f"lat{lc}") for lc in range(4)]
                        pkr = PSL.tile([64, 512], F32, tag="lat0_1",
                                       name="pkr")
                        for hch in range(8):
                            if tq == 0:
                                nc.sync.dma_start(wdkv[hch][:], wd_kvT_r2[hch])
                                nc.sync.dma_start(wkr[hch][:, :, 0:R],
                                                  wkrT_r2[hch])
                                nc.sync.dma_start(wkr[hch][:, :, R:2 * R],
                                                  wkrrT_r2[hch])
                            xbt = XS.tile([128, 2, 512], BF16, tag="xb",
                                          name="xb")
                            nc.sync.dma_start(xbt[:], xbT_r2[hch][:, :, tsl])
                            for two in range(2):
                                for lc in range(4):
                                    nc.tensor.matmul(
                                        pss[lc][:],
                                        lhsT=wdkv[hch][:, two,
                                                       lc * 128:(lc + 1) * 128],
                                        rhs=xbt[:, two, :],
                                        start=(hch == 0 and two == 0),
                                        stop=(hch == 7 and two == 1),
                                    )
                                nc.tensor.matmul(
                                    pkr[:],
                                    lhsT=wkr[hch][:, two, :],
                                    rhs=xbt[:, two, :],
                                    start=(hch == 0 and two == 0),
                                    stop=(hch == 7 and two == 1),
                                )
                        if tq == 0:
                            nc.sync.dma_start(coskT[:], coskT_d[:, :])
                            nc.sync.dma_start(sinkT[:], sinkT_d[:, :])
                        for lc in range(4):
                            nc.scalar.copy(kv_latT[:, lc, tsl], pss[lc][:])
                        t1 = T1.tile([32, 512], F32, tag="r1", name="r1")
                        t2_ = T1.tile([32, 512], F32, tag="r2", name="r2")
                        nc.vector.tensor_mul(t1[:], pkr[0:32, :], coskT[:, tsl])
                        nc.vector.tensor_mul(t2_[:], pkr[32:64, :],
                                             sinkT[:, tsl])
                        nc.vector.tensor_add(krT[:, tsl], t1[:], t2_[:])

                    # q_latT[l, q] over the core's own query chunk
                    psq = [PSL.tile([128, 512], F32, tag=f"lat{lc}_0",
                                    name=f"q{lc}") for lc in range(4)]
                    for hch in range(8):
                        nc.sync.dma_start(wdq[hch][:], wd_qT_r2[hch])
                        xqt = XS.tile([128, 2, SQ], BF16, tag="xq", name="xq")
                        nc.sync.dma_start(xqt[:], xqT_r2[hch])
                        for two in range(2):
                            for lc in range(4):
                                nc.tensor.matmul(
                                    psq[lc][:],
                                    lhsT=wdq[hch][:, two,
                                                  lc * 128:(lc + 1) * 128],
                                    rhs=xqt[:, two, :],
                                    start=(hch == 0 and two == 0),
                                    stop=(hch == 7 and two == 1),
                                )
                    for lc in range(4):
                        nc.scalar.copy(q_latT[:, lc, :], psq[lc][:])

                # ------------ Phase 2: per head-group proj + attention ----
                nc.sync.dma_start(maskT[:], maskT_d[:, :])

                with tc.tile_pool(name="grp", bufs=2) as GRP, \
                     tc.tile_pool(name="gw", bufs=2) as GW, \
                     tc.tile_pool(name="et", bufs=2) as ETP, \
                     tc.tile_pool(name="tmp2", bufs=2) as T2, \
                     tc.tile_pool(name="rcp", bufs=2) as RCP, \
                     tc.tile_pool(name="ps_p", bufs=1, space="PSUM") as PSP, \
                     tc.tile_pool(name="ps_s", bufs=3, space="PSUM") as PSS, \
                     tc.tile_pool(name="ps_c", bufs=2, space="PSUM") as PSC, \
                     tc.tile_pool(name="ps_r", bufs=1, space="PSUM") as PSR:
                    for g in range(NG):
                        wk_g = GW.tile([128, 4, GH * C], BF16, tag="wk", name="wk")
                        wv_g = GW.tile([128, 4, GH * DH], BF16, tag="wv", name="wv")
                        wq_g = GW.tile([128, 4, GH * 128], BF16, tag="wq", name="wq")
                        wqr_g = GW.tile([128, 4, GH * 2 * R], BF16, tag="wqr", name="wqr")
                        for lc in range(4):
                            nc.sync.dma_start(
                                wk_g[:, lc, :],
                                wk_upT_r[lc][:, g * GH * C:(g + 1) * GH * C],
                            )
                            nc.sync.dma_start(
                                wv_g[:, lc, :],
                                wv_upT_r[lc][:, g * GH * DH:(g + 1) * GH * DH],
                            )
                            nc.sync.dma_start(
                                wq_g[:, lc, :],
                                wqcrT_r[lc][:, g * GH * 128:(g + 1) * GH * 128],
                            )
                            nc.sync.dma_start(
                                wqr_g[:, lc, :],
                                wqrrT_r[lc][:, g * GH * 2 * R:(g + 1) * GH * 2 * R],
                            )

                        kT_g = GRP.tile([128, GH, S], BF16, tag="kT", name="kT")
                        v_g = GRP.tile([128, NKC, GH * DH], BF16, tag="vG", name="vG")
                        qT_g = GRP.tile([128, GH, SQ], BF16, tag="qT", name="qT")

                        # k content rows [0:96], shared roped k_rope rows [96:128]
                        for hh in range(GH):
                            for half in range(2):
                                pk = [PSP.tile([128, 512], F32, tag=("pa", "pb")[t2],
                                               name=f"pk{t2}")
                                      for t2 in range(2)]
                                for lc in range(4):
                                    for t2 in range(2):
                                        nc.tensor.matmul(
                                            pk[t2][0:C, :],
                                            lhsT=wk_g[:, lc, hh * C:(hh + 1) * C],
                                            rhs=kv_latT[
                                                :, lc,
                                                (half * 2 + t2) * 512:
                                                (half * 2 + t2 + 1) * 512,
                                            ],
                                            start=(lc == 0), stop=(lc == 3),
                                        )
                                for t2 in range(2):
                                    t0 = (half * 2 + t2) * 512
                                    nc.scalar.copy(
                                        kT_g[0:C, hh, t0:t0 + 512],
                                        pk[t2][0:C, :],
                                    )
                            nc.sync.dma_start(kT_g[C:128, hh, :], krT[:, :])

                        # v[t, d] for the group 4 heads
                        for kc in range(NKC):
                            pv = PSP.tile([128, 512], F32, tag="pa", name="pv")
                            for lc in range(4):
                                nc.tensor.matmul(
                                    pv[:],
                                    lhsT=kv_latT[:, lc, kc * 128:(kc + 1) * 128],
                                    rhs=wv_g[:, lc, :],
                                    start=(lc == 0), stop=(lc == 3),
                                )
                            nc.scalar.copy(v_g[:, kc, :], pv[:])

                        # q: content + roped rope rows
                        for hh in range(GH):
                            pqc = PSP.tile([96, 512], F32, tag="pa", name="pqc")
                            pqr2 = PSP.tile([64, 512], F32, tag="pb", name="pqr2")
                            for lc in range(4):
                                nc.tensor.matmul(
                                    pqc[:],
                                    lhsT=wq_g[:, lc, hh * 128:hh * 128 + C],
                                    rhs=q_latT[:, lc, :],
                                    start=(lc == 0), stop=(lc == 3),
                                )
                                nc.tensor.matmul(
                                    pqr2[:],
                                    lhsT=wqr_g[:, lc, hh * 2 * R:(hh + 1) * 2 * R],
                                    rhs=q_latT[:, lc, :],
                                    start=(lc == 0), stop=(lc == 3),
                                )
                            nc.scalar.copy(qT_g[0:C, hh, :], pqc[:])
                            t1 = T2.tile([32, SQ], F32, tag="r1", name="t1")
                            t2_ = T2.tile([32, SQ], F32, tag="r2", name="t2")
                            t3 = T2.tile([32, SQ], BF16, tag="r3", name="t3")
                            nc.vector.tensor_mul(t1[:], pqr2[0:32, :], cosqT[:])
                            nc.vector.tensor_mul(t2_[:], pqr2[32:64, :], sinqT[:])
                            nc.vector.tensor_add(t3[:], t1[:], t2_[:])
                            nc.sync.dma_start(qT_g[C:128, hh, :], t3[:])

                        # attention for the group heads
                        for hh in range(GH):
                            h = g * GH + hh
                            ets = []
                            for kc in range(NKC):
                                ps = PSS.tile([128, 512], F32, tag="s",
                                              name=f"ps{kc % 2}")
                                nc.tensor.matmul(
                                    ps[:],
                                    lhsT=kT_g[:, hh, kc * 128:(kc + 1) * 128],
                                    rhs=qT_g[:, hh, :],
                                    start=True, stop=True,
                                )
                                nc.vector.tensor_add(
                                    ps[:], ps[:],
                                    maskT[:, kc * SQ:(kc + 1) * SQ],
                                )
                                et = ETP.tile([128, SQ], BF16, tag=f"e{kc}",
                                              name=f"et{kc}")
                                nc.scalar.activation(
                                    et[:], ps[:],
                                    mybir.ActivationFunctionType.Exp,
                                )
                                ets.append(et)
                            pctx = PSC.tile([128, 512], F32, tag="c",
                                            name=f"pctx{hh % 2}")
                            prs = PSR.tile([128, 512], F32, tag="r",
                                           name=f"prs{hh % 2}")
                            for kc in range(NKC):
                                nc.tensor.matmul(
                                    pctx[:],
                                    lhsT=v_g[:, kc, hh * DH:(hh + 1) * DH],
                                    rhs=ets[kc][:],
                                    start=(kc == 0), stop=(kc == NKC - 1),
                                )
                            # in-place DVE tree-sum of the exp tiles,
                            # then a single all-ones matmul for the row-sum
                            step = 1
                            while step < NKC:
                                for i in range(0, NKC, 2 * step):
                                    nc.vector.tensor_add(
                                        ets[i][:], ets[i][:], ets[i + step][:]
                                    )
                                step *= 2
                            nc.tensor.matmul(
                                prs[:], lhsT=onesb[:], rhs=ets[0][:],
                                start=True, stop=True,
                            )
                            rc = RCP.tile([128, 512], F32, tag="rc",
                                          name=f"rc{hh % 2}")
                            nc.vector.reciprocal_approx_fast(out=rc[:], in_=prs[:])
                            nc.vector.tensor_mul(ctxT[:, h, :], pctx[:], rc[:])

            # ---------------- Phase 3: output projection ------------------
            with tc.tile_pool(name="op", bufs=2) as OP, \
                 tc.tile_pool(name="ps_o", bufs=2, space="PSUM") as PSO:
                WO = tc.alloc_tile_pool(name="wo", bufs=1, side="right")
                wo_sb = [WO.tile([128, HID], BF16, tag=f"wo{hc}",
                                 name=f"wo{hc}") for hc in range(16)]
                bo_sb = WO.tile([128, HID], F32, tag="bo", name="bo")
                nc.sync.dma_start(bo_sb[:], bo_d[:, :])
                for hc in range(16):
                    nc.sync.dma_start(wo_sb[hc][:], woT_r[hc])
                for tq in range(4):
                    pos = [PSO.tile([128, 512], F32, tag=f"o{oc}",
                                    name=f"pos{oc}") for oc in range(4)]
                    for h in range(16):
                        for oc in range(4):
                            nc.tensor.matmul(
                                pos[oc][:],
                                lhsT=ctxT[:, h, tq * 128:(tq + 1) * 128],
                                rhs=wo_sb[h][:, oc * 512:(oc + 1) * 512],
                                start=(h == 0), stop=(h == 15),
                            )
                    ot = OP.tile([128, HID], F32, tag="ot", name="ot")
                    for oc in range(4):
                        nc.vector.tensor_add(
                            ot[:, oc * 512:(oc + 1) * 512],
                            pos[oc][:],
                            bo_sb[:, oc * 512:(oc + 1) * 512],
                        )
                    nc.sync.dma_start(
                        out_d[tq * 128:(tq + 1) * 128, :], ot[:]
                    )
                WO.release()

    nc.compile()
    return nc


def _rot_rows(w):
    # rows of w are the rope dim; rot(w) @ lat == rotate_half(w @ lat)
    hR = w.shape[0] // 2
    return np.concatenate([-w[hR:], w[:hR]], axis=0)


def _prep_inputs(inputs):
    x = np.asarray(inputs["x"], np.float32)
    Wq_down = np.asarray(inputs["Wq_down"], np.float32)
    Wq_up = np.asarray(inputs["Wq_up"], np.float32)
    Wq_rope = np.asarray(inputs["Wq_rope"], np.float32)
    Wkv_down = np.asarray(inputs["Wkv_down"], np.float32)
    Wk_up = np.asarray(inputs["Wk_up"], np.float32)
    Wk_rope = np.asarray(inputs["Wk_rope"], np.float32)
    Wv_up = np.asarray(inputs["Wv_up"], np.float32)
    Wo = np.asarray(inputs["Wo"], np.float32)
    bo = np.asarray(inputs["bo"], np.float32)

    s = np.float32(1.0 / np.sqrt(DH))

    wd_kvT = np.ascontiguousarray(Wkv_down.T).astype(BF16NP)
    wd_qT = np.ascontiguousarray(Wq_down.T).astype(BF16NP)
    wkrT = np.ascontiguousarray(Wk_rope.T).astype(BF16NP)
    wkrrT = np.ascontiguousarray(_rot_rows(Wk_rope).T).astype(BF16NP)
    wk_upT = np.ascontiguousarray(Wk_up.T).astype(BF16NP)
    wv_upT = np.ascontiguousarray(Wv_up.T).astype(BF16NP)

    wqcr = np.empty((128 * H, LAT), np.float32)
    wqrr = np.empty((2 * R * H, LAT), np.float32)
    for h in range(H):
        wqcr[h * 128:h * 128 + C] = Wq_up[h * C:(h + 1) * C] * s
        wqcr[h * 128 + C:(h + 1) * 128] = Wq_rope[h * R:(h + 1) * R] * s
        wqrr[h * 2 * R:h * 2 * R + R] = Wq_rope[h * R:(h + 1) * R] * s
        wqrr[h * 2 * R + R:(h + 1) * 2 * R] = _rot_rows(Wq_rope[h * R:(h + 1) * R]) * s
    wqcrT = np.ascontiguousarray(wqcr.T).astype(BF16NP)
    wqrrT = np.ascontiguousarray(wqrr.T).astype(BF16NP)
    woT = np.ascontiguousarray(Wo.T).astype(BF16NP)
    bo2 = np.ascontiguousarray(np.broadcast_to(bo.reshape(1, HID), (128, HID)))

    inv_freq = (1.0 / (10000.0 ** (np.arange(0, R, 2, dtype=np.float32) / R)))
    t = np.arange(S, dtype=np.float32)
    freqs = t[:, None] * inv_freq[None, :]
    emb = np.concatenate([freqs, freqs], axis=-1)          # [S, R]
    cos = np.cos(emb).astype(np.float32)
    sin = np.sin(emb).astype(np.float32)
    coskT = np.ascontiguousarray(cos.T)
    sinkT = np.ascontiguousarray(sin.T)

    kar = np.arange(128)[:, None]
    qar = np.arange(SQ)[None, :]

    in_maps = []
    for c in range(8):
        b, j = divmod(c, 4)
        q0 = j * SQ
        maskT = np.empty((128, NKC * SQ), np.float32)
        for kc in range(NKC):
            vis = (kc * 128 + kar) <= (q0 + qar)
            maskT[:, kc * SQ:(kc + 1) * SQ] = np.where(vis, 0.0, -10000.0)
        in_maps.append({
            "xbT": np.ascontiguousarray(x[b].T).astype(BF16NP),
            "xqT": np.ascontiguousarray(x[b, q0:q0 + SQ].T).astype(BF16NP),
            "wd_kvT": wd_kvT, "wd_qT": wd_qT,
            "wkrT": wkrT, "wkrrT": wkrrT,
            "wk_upT": wk_upT, "wv_upT": wv_upT,
            "wqcrT": wqcrT, "wqrrT": wqrrT,
            "woT": woT, "bo": bo2,
            "cosqT": np.ascontiguousarray(cos[q0:q0 + SQ].T),
            "sinqT": np.ascontiguousarray(sin[q0:q0 + SQ].T),
            "coskT": coskT.astype(BF16NP), "sinkT": sinkT.astype(BF16NP),
            "maskT": maskT.astype(BF16NP),
        })
    return in_maps


_NC_CACHE = None


def kernel(**inputs):
    global _NC_CACHE
    if _NC_CACHE is None:
        _NC_CACHE = build_nc()
    nc = _NC_CACHE
    in_maps = _prep_inputs(inputs)
    res = run_bass_kernel_spmd(nc, in_maps, list(range(8)))
    out = np.empty((B, S, HID), np.float32)
    for c in range(8):
        b, j = divmod(c, 4)
        out[b, j * SQ:(j + 1) * SQ] = res.results[c]["out"]
    return out

